# revision 1
# baseline (speedup 1.0000x reference)
"""Multi-LoRA batched low-rank adapter kernel for 8 trn2 NeuronCores.

Problem: x [16, 2048, 4096] f32, adapter_ids [16] int, A [64, 4096, 64],
B [64, 64, 4096].  out[b] = (x[b] @ B[id_b].T) @ A[id_b].T * (1/64).

Sharding: data-parallel over batch (2 samples/core); the per-sample
adapters are gathered on host (adapter_ids are host-visible and tiny)
and x is pre-transposed on host so the mm1 contraction dim lands on
SBUF partitions.

Numerics: every operand X is split on host into X_hi = bf16(X) and
X_lo = bf16(X - X_hi) (~16-bit effective mantissa).  Each matmul is
computed as 3 bf16 terms (hi*hi + hi*lo + lo*hi), accumulated in fp32
PSUM: rel error ~1e-5 at ~4x the fp32r PE rate measured on this HW
(194ns vs 747ns per 512-col matmul).

Rank (64) is zero-padded to 128 so mm2's contraction K=128 keeps the
fast-weight-load path enabled (K=64 measured +200ns/matmul).

PSUM drains: [128,1024] 2-bank copies (fixed ~600ns/op overhead
amortized), split between DVE and ACT.  Loads issue on the SP HWDGE
ring, stores on gpsimd (SWDGE) so drain engines never head-of-line
block a ring.  Samples are software-pipelined: mm2 of sample s is
interleaved with mm1 of sample s+1 so PE/DMA/drain all stay busy.
"""

import numpy as np
from contextlib import ExitStack

import concourse.bass as bass
import concourse.tile as tile
from concourse import bacc, mybir, bass_utils

NCORES = 8
BATCH = 16
B_PER = BATCH // NCORES
SEQ = 2048
DIN = 4096
DOUT = 4096
RANK = 64
RPAD = 128
SCALE = np.float32(1.0 / 64.0)

f32 = mybir.dt.float32
bf16 = mybir.dt.bfloat16

P = 128
KI = DIN // P      # 32 contraction tiles for mm1
KK = 4             # k-tiles per x slab DMA
SLABS = KI // KK   # 8
NB = SEQ // 512    # 4
NSUB = SEQ // P    # 16
OT = DOUT // 512   # 8

_CACHE = {}


def _build_nc(repeat=1):
    nc = bacc.Bacc("TRN2", target_bir_lowering=False, debug=False)
    xh_d = nc.dram_tensor("xh", [B_PER, DIN, SEQ], bf16, kind="ExternalInput").ap()
    xl_d = nc.dram_tensor("xl", [B_PER, DIN, SEQ], bf16, kind="ExternalInput").ap()
    bh_d = nc.dram_tensor("bh", [B_PER, DIN, RPAD], bf16, kind="ExternalInput").ap()
    bl_d = nc.dram_tensor("bl", [B_PER, DIN, RPAD], bf16, kind="ExternalInput").ap()
    ah_d = nc.dram_tensor("ah", [B_PER, RPAD, DOUT], bf16, kind="ExternalInput").ap()
    al_d = nc.dram_tensor("al", [B_PER, RPAD, DOUT], bf16, kind="ExternalInput").ap()
    out = nc.dram_tensor("out", [B_PER, SEQ, DOUT], f32, kind="ExternalOutput").ap()

    with tile.TileContext(nc) as tc, ExitStack() as ctx:
        adp = ctx.enter_context(tc.tile_pool(name="adp", bufs=2))
        xhp = ctx.enter_context(tc.tile_pool(name="xhp", bufs=2))
        xlp = ctx.enter_context(tc.tile_pool(name="xlp", bufs=2))
        bxsp = ctx.enter_context(tc.tile_pool(name="bxsp", bufs=2))
        stg = ctx.enter_context(tc.tile_pool(name="stg", bufs=3))
        bxp = ctx.enter_context(tc.tile_pool(name="bxp", bufs=NB, space="PSUM"))
        outp = ctx.enter_context(tc.tile_pool(name="outp", bufs=2, space="PSUM"))

        def load_adapters(s):
            ad = {}
            for nm, dram in (("bh", bh_d), ("bl", bl_d)):
                t = adp.tile([P, KI, RPAD], bf16, name=nm, tag=nm)
                nc.sync.dma_start(t[:], dram[s].rearrange("(k p) r -> p k r", p=P))
                ad[nm] = t
            for nm, dram in (("ah", ah_d), ("al", al_d)):
                t = adp.tile([RPAD, DOUT], bf16, name=nm, tag=nm)
                nc.sync.dma_start(t[:], dram[s])
                ad[nm] = t
            return ad

        def mm1_slab(s, j, ad, bx_ps):
            """Load x slab j (hi+lo) and run its mm1 matmuls."""
            xht = xhp.tile([P, KK, SEQ], bf16, name="xht", tag="xht")
            nc.sync.dma_start(
                xht[:], xh_d[s, j * KK * P:(j + 1) * KK * P, :].rearrange(
                    "(kk p) n -> p kk n", p=P))
            xlt = xlp.tile([P, KK, SEQ], bf16, name="xlt", tag="xlt")
            nc.sync.dma_start(
                xlt[:], xl_d[s, j * KK * P:(j + 1) * KK * P, :].rearrange(
                    "(kk p) n -> p kk n", p=P))
            for kk in range(KK):
                k = j * KK + kk
                for nb in range(NB):
                    mv = slice(nb * 512, (nb + 1) * 512)
                    first = (k == 0)
                    last = (k == KI - 1)
                    nc.tensor.matmul(bx_ps[nb][:], ad["bh"][:, k, :],
                                     xht[:, kk, mv], start=first, stop=False)
                    nc.tensor.matmul(bx_ps[nb][:], ad["bl"][:, k, :],
                                     xht[:, kk, mv], start=False, stop=False)
                    nc.tensor.matmul(bx_ps[nb][:], ad["bh"][:, k, :],
                                     xlt[:, kk, mv], start=False, stop=last)

        def bx_split(bx_ps):
            """Drain mm1 PSUM to bf16 hi/lo SBUF pair."""
            bxh = bxsp.tile([RPAD, SEQ], bf16, name="bxh", tag="bxh")
            bxl = bxsp.tile([RPAD, SEQ], bf16, name="bxl", tag="bxl")
            for nb in range(NB):
                sl = slice(nb * 512, (nb + 1) * 512)
                nc.vector.tensor_copy(bxh[:, sl], bx_ps[nb][:])
                nc.vector.tensor_sub(bxl[:, sl], bx_ps[nb][:], bxh[:, sl])
            return bxh, bxl

        def mm2_block(s, ns, ad, bxh, bxl):
            """One 128-row output block: 8 ot matmul-triples + drains + store."""
            st = stg.tile([P, DOUT], f32, name="st", tag="st")
            for otp in range(OT // 2):  # pairs of 512-col blocks
                ps = outp.tile([P, 1024], f32, name="ps_o", tag="ps_o")
                for half in range(2):
                    ot = otp * 2 + half
                    ov = slice(ot * 512, (ot + 1) * 512)
                    pv = slice(half * 512, (half + 1) * 512)
                    lh = slice(ns * P, (ns + 1) * P)
                    nc.tensor.matmul(ps[:, pv], bxh[:, lh], ad["ah"][:, ov],
                                     start=True, stop=False)
                    nc.tensor.matmul(ps[:, pv], bxh[:, lh], ad["al"][:, ov],
                                     start=False, stop=False)
                    nc.tensor.matmul(ps[:, pv], bxl[:, lh], ad["ah"][:, ov],
                                     start=False, stop=True)
                dv = slice(otp * 1024, (otp + 1) * 1024)
                if otp % 2 == 0:
                    nc.vector.tensor_copy(st[:, dv], ps[:])
                else:
                    nc.scalar.copy(st[:, dv], ps[:])
            nc.gpsimd.dma_start(out[s, ns * P:(ns + 1) * P, :], st[:])

        def mm1_sample(s, ad):
            bx_ps = [bxp.tile([P, 512], f32, name="bx_ps", tag="bx_ps")
                     for _ in range(NB)]
            for j in range(SLABS):
                mm1_slab(s, j, ad, bx_ps)
            return bx_split(bx_ps)

        samples = [s for _ in range(repeat) for s in range(B_PER)]
        # software pipeline: mm1(s0); then interleave mm2(s_i) with mm1(s_{i+1})
        ad_cur = load_adapters(samples[0])
        bxh, bxl = mm1_sample(samples[0], ad_cur)
        for idx, s in enumerate(samples):
            nxt = samples[idx + 1] if idx + 1 < len(samples) else None
            if nxt is not None:
                ad_nxt = load_adapters(nxt)
                bx_ps_n = [bxp.tile([P, 512], f32, name="bx_ps", tag="bx_ps")
                           for _ in range(NB)]
                for ns in range(NSUB):
                    mm2_block(s, ns, ad_cur, bxh, bxl)
                    if ns % 2 == 0:
                        mm1_slab(nxt, ns // 2, ad_nxt, bx_ps_n)
                bxh, bxl = bx_split(bx_ps_n)
                ad_cur = ad_nxt
            else:
                for ns in range(NSUB):
                    mm2_block(s, ns, ad_cur, bxh, bxl)
    nc.compile()
    return nc


def _get_nc(repeat=1):
    key = f"nc{repeat}"
    if key not in _CACHE:
        _CACHE[key] = _build_nc(repeat)
    return _CACHE[key]


def _split(a):
    import ml_dtypes
    hi = a.astype(ml_dtypes.bfloat16)
    lo = (a - hi.astype(np.float32)).astype(ml_dtypes.bfloat16)
    return hi, lo


def _prep_in_maps(x, adapter_ids, A, B):
    x = np.asarray(x, dtype=np.float32)
    ids = np.asarray(adapter_ids).astype(np.int64)
    A = np.asarray(A, dtype=np.float32)
    B = np.asarray(B, dtype=np.float32)

    As = A * SCALE
    in_maps = []
    for c in range(NCORES):
        sl = slice(c * B_PER, (c + 1) * B_PER)
        cids = ids[sl]
        xT = np.ascontiguousarray(x[sl].transpose(0, 2, 1))       # [2, DIN, SEQ]
        BT = np.zeros((B_PER, DIN, RPAD), np.float32)
        BT[:, :, :RANK] = B[cids].transpose(0, 2, 1)
        AT = np.zeros((B_PER, RPAD, DOUT), np.float32)
        AT[:, :RANK, :] = As[cids].transpose(0, 2, 1)
        xh, xl = _split(xT)
        bh, bl = _split(BT)
        ah, al = _split(AT)
        in_maps.append({"xh": xh, "xl": xl, "bh": bh, "bl": bl,
                        "ah": ah, "al": al})
    return in_maps


def kernel(x, adapter_ids, A, B):
    nc = _get_nc()
    in_maps = _prep_in_maps(x, adapter_ids, A, B)
    res = bass_utils.run_bass_kernel_spmd(
        nc, in_maps, core_ids=list(range(NCORES)))
    out = np.empty((BATCH, SEQ, DOUT), dtype=np.float32)
    for c in range(NCORES):
        out[c * B_PER:(c + 1) * B_PER] = res.results[c]["out"]
    return out



# revision 2
# speedup vs baseline: 1.5823x; 1.5823x over previous
"""Multi-LoRA batched low-rank adapter kernel for 8 trn2 NeuronCores.

Problem: x [16, 2048, 4096] f32, adapter_ids [16] int, A [64, 4096, 64],
B [64, 64, 4096].  out[b] = (x[b] @ B[id_b].T) @ A[id_b].T * (1/64).

Sharding: data-parallel over batch (2 samples/core); the per-sample
adapters are gathered on host (adapter_ids are host-visible and tiny)
and x is pre-transposed on host so the mm1 contraction dim lands on
SBUF partitions.

The workload is HBM-bound (~358 GB/s/core ceiling, shared HBM stack),
so all operands and the output travel as fp16 (measured end-to-end rel
err ~5e-4 vs the fp32 reference, well inside the 2e-2 gate; fp16
matmul runs at the same PE rate as bf16).  The output is upcast to
fp32 on host.  Traffic/core: 32 MB x + 32 MB out + 6 MB adapters,
~190 us at the HBM ceiling vs ~110 us of PE time.

Rank (64) is zero-padded to 128 so mm2's contraction K=128 keeps the
fast-weight-load path enabled (K=64 measured +200ns/matmul).

PSUM drains: [128,1024] 2-bank copies (fixed ~600ns/op overhead
amortized), split between DVE and ACT.  Loads issue on the SP HWDGE
ring, stores on gpsimd (SWDGE) so drain engines never head-of-line
block a ring.  Samples are software-pipelined: mm2 of sample s is
interleaved with mm1 of sample s+1 so PE/DMA/drain all stay busy.
"""

import numpy as np
from contextlib import ExitStack

import concourse.bass as bass
import concourse.tile as tile
from concourse import bacc, mybir, bass_utils

NCORES = 8
BATCH = 16
B_PER = BATCH // NCORES
SEQ = 2048
DIN = 4096
DOUT = 4096
RANK = 64
RPAD = 128
SCALE = np.float32(1.0 / 64.0)

f32 = mybir.dt.float32
f16 = mybir.dt.float16

P = 128
KI = DIN // P      # 32 contraction tiles for mm1
KK = 4             # k-tiles per x slab DMA
SLABS = KI // KK   # 8
NB = SEQ // 512    # 4
NSUB = SEQ // P    # 16
OT = DOUT // 512   # 8

_CACHE = {}


def _build_nc(repeat=1):
    nc = bacc.Bacc("TRN2", target_bir_lowering=False, debug=False)
    xh_d = nc.dram_tensor("xh", [B_PER, DIN, SEQ], f16, kind="ExternalInput").ap()
    bh_d = nc.dram_tensor("bh", [B_PER, DIN, RPAD], f16, kind="ExternalInput").ap()
    ah_d = nc.dram_tensor("ah", [B_PER, RPAD, DOUT], f16, kind="ExternalInput").ap()
    out = nc.dram_tensor("out", [B_PER, SEQ, DOUT], f16, kind="ExternalOutput").ap()

    with tile.TileContext(nc) as tc, ExitStack() as ctx:
        adp = ctx.enter_context(tc.tile_pool(name="adp", bufs=2))
        xhp = ctx.enter_context(tc.tile_pool(name="xhp", bufs=4))
        bxsp = ctx.enter_context(tc.tile_pool(name="bxsp", bufs=2))
        stg = ctx.enter_context(tc.tile_pool(name="stg", bufs=4))
        bxp = ctx.enter_context(tc.tile_pool(name="bxp", bufs=NB, space="PSUM"))
        outp = ctx.enter_context(tc.tile_pool(name="outp", bufs=2, space="PSUM"))

        def load_adapters(s):
            ad = {}
            bht = adp.tile([P, KI, RPAD], f16, name="bh", tag="bh")
            nc.sync.dma_start(bht[:], bh_d[s].rearrange("(k p) r -> p k r", p=P))
            ad["bh"] = bht
            aht = adp.tile([RPAD, DOUT], f16, name="ah", tag="ah")
            nc.sync.dma_start(aht[:], ah_d[s])
            ad["ah"] = aht
            return ad

        def mm1_slab(s, j, ad, bx_ps):
            """Load x slab j and run its mm1 matmuls."""
            xht = xhp.tile([P, KK, SEQ], f16, name="xht", tag="xht")
            nc.sync.dma_start(
                xht[:], xh_d[s, j * KK * P:(j + 1) * KK * P, :].rearrange(
                    "(kk p) n -> p kk n", p=P))
            for kk in range(KK):
                k = j * KK + kk
                for nb in range(NB):
                    mv = slice(nb * 512, (nb + 1) * 512)
                    nc.tensor.matmul(bx_ps[nb][:], ad["bh"][:, k, :],
                                     xht[:, kk, mv],
                                     start=(k == 0), stop=(k == KI - 1))

        def bx_drain(bx_ps):
            """Drain mm1 PSUM to fp16 SBUF."""
            bxh = bxsp.tile([RPAD, SEQ], f16, name="bxh", tag="bxh")
            for nb in range(NB):
                sl = slice(nb * 512, (nb + 1) * 512)
                if nb % 2 == 0:
                    nc.vector.tensor_copy(bxh[:, sl], bx_ps[nb][:])
                else:
                    nc.scalar.copy(bxh[:, sl], bx_ps[nb][:])
            return bxh

        def mm2_block(s, ns, ad, bxh):
            """One 128-row output block: 8 ot matmuls + drains + store."""
            st = stg.tile([P, DOUT], f16, name="st", tag="st")
            lh = slice(ns * P, (ns + 1) * P)
            for otp in range(OT // 2):  # pairs of 512-col blocks
                ps = outp.tile([P, 1024], f32, name="ps_o", tag="ps_o")
                for half in range(2):
                    ot = otp * 2 + half
                    ov = slice(ot * 512, (ot + 1) * 512)
                    pv = slice(half * 512, (half + 1) * 512)
                    nc.tensor.matmul(ps[:, pv], bxh[:, lh], ad["ah"][:, ov],
                                     start=True, stop=True)
                dv = slice(otp * 1024, (otp + 1) * 1024)
                if otp % 2 == 0:
                    nc.vector.tensor_copy(st[:, dv], ps[:])
                else:
                    nc.scalar.copy(st[:, dv], ps[:])
            nc.gpsimd.dma_start(out[s, ns * P:(ns + 1) * P, :], st[:])

        def mm1_sample(s, ad):
            bx_ps = [bxp.tile([P, 512], f32, name="bx_ps", tag="bx_ps")
                     for _ in range(NB)]
            for j in range(SLABS):
                mm1_slab(s, j, ad, bx_ps)
            return bx_drain(bx_ps)

        samples = [s for _ in range(repeat) for s in range(B_PER)]
        # software pipeline: mm1(s0); then interleave mm2(s_i) with mm1(s_{i+1})
        ad_cur = load_adapters(samples[0])
        bxh = mm1_sample(samples[0], ad_cur)
        for idx, s in enumerate(samples):
            nxt = samples[idx + 1] if idx + 1 < len(samples) else None
            if nxt is not None:
                ad_nxt = load_adapters(nxt)
                bx_ps_n = [bxp.tile([P, 512], f32, name="bx_ps", tag="bx_ps")
                           for _ in range(NB)]
                for ns in range(NSUB):
                    mm2_block(s, ns, ad_cur, bxh)
                    if ns % 2 == 0:
                        mm1_slab(nxt, ns // 2, ad_nxt, bx_ps_n)
                bxh = bx_drain(bx_ps_n)
                ad_cur = ad_nxt
            else:
                for ns in range(NSUB):
                    mm2_block(s, ns, ad_cur, bxh)
    nc.compile()
    return nc


def _get_nc(repeat=1):
    key = f"nc{repeat}"
    if key not in _CACHE:
        _CACHE[key] = _build_nc(repeat)
    return _CACHE[key]


def _prep_in_maps(x, adapter_ids, A, B):
    x = np.asarray(x, dtype=np.float32)
    ids = np.asarray(adapter_ids).astype(np.int64)
    A = np.asarray(A, dtype=np.float32)
    B = np.asarray(B, dtype=np.float32)

    As = A * SCALE
    in_maps = []
    for c in range(NCORES):
        sl = slice(c * B_PER, (c + 1) * B_PER)
        cids = ids[sl]
        xT = np.ascontiguousarray(
            x[sl].transpose(0, 2, 1)).astype(np.float16)          # [2, DIN, SEQ]
        BT = np.zeros((B_PER, DIN, RPAD), np.float16)
        BT[:, :, :RANK] = B[cids].transpose(0, 2, 1)
        AT = np.zeros((B_PER, RPAD, DOUT), np.float16)
        AT[:, :RANK, :] = As[cids].transpose(0, 2, 1)
        in_maps.append({"xh": xT, "bh": BT, "ah": AT})
    return in_maps


def kernel(x, adapter_ids, A, B):
    nc = _get_nc()
    in_maps = _prep_in_maps(x, adapter_ids, A, B)
    res = bass_utils.run_bass_kernel_spmd(
        nc, in_maps, core_ids=list(range(NCORES)))
    out = np.empty((BATCH, SEQ, DOUT), dtype=np.float32)
    for c in range(NCORES):
        out[c * B_PER:(c + 1) * B_PER] = res.results[c]["out"].astype(np.float32)
    return out


# revision 4
# speedup vs baseline: 2.0807x; 1.3150x over previous
"""Multi-LoRA batched low-rank adapter kernel for 8 trn2 NeuronCores.

Problem: x [16, 2048, 4096] f32, adapter_ids [16] int, A [64, 4096, 64],
B [64, 64, 4096].  out[b] = (x[b] @ B[id_b].T) @ A[id_b].T * (1/64).

Sharding: data-parallel over batch (2 samples/core); the per-sample
adapters are gathered on host (adapter_ids are host-visible and tiny)
and x is pre-transposed on host so the mm1 contraction dim lands on
SBUF partitions.

The workload is HBM-bound (~358 GB/s/core ceiling, shared HBM stack),
so all operands and the output travel as fp16 (measured end-to-end rel
err ~5e-4 vs the fp32 reference, well inside the 2e-2 gate; fp16
matmul runs at the same PE rate as bf16).  The output is upcast to
fp32 on host.  Traffic/core: 32 MB x + 32 MB out + 6 MB adapters,
~190 us at the HBM ceiling vs ~110 us of PE time.

Rank (64) is zero-padded to 128 so mm2's contraction K=128 keeps the
fast-weight-load path enabled (K=64 measured +200ns/matmul).

PSUM drains: [128,1024] 2-bank copies (fixed ~600ns/op overhead
amortized), split between DVE and ACT.  Loads issue on the SP HWDGE
ring, stores on gpsimd (SWDGE) so drain engines never head-of-line
block a ring.  Samples are software-pipelined: mm2 of sample s is
interleaved with mm1 of sample s+1 so PE/DMA/drain all stay busy.
"""

import numpy as np
from contextlib import ExitStack

import concourse.bass as bass
import concourse.tile as tile
from concourse import bacc, mybir, bass_utils

NCORES = 8
BATCH = 16
B_PER = BATCH // NCORES
SEQ = 2048
DIN = 4096
DOUT = 4096
RANK = 64
RPAD = 128
SCALE = np.float32(1.0 / 64.0)

f32 = mybir.dt.float32
f16 = mybir.dt.float16

P = 128
KI = DIN // P      # 32 contraction tiles for mm1
KK = 4             # k-tiles per x slab DMA
SLABS = KI // KK   # 8
NB = SEQ // 512    # 4
NSUB = SEQ // P    # 16
OT = DOUT // 512   # 8

_CACHE = {}


def _build_nc(repeat=1):
    nc = bacc.Bacc("TRN2", target_bir_lowering=False, debug=False)
    # Host pre-arranges every tensor so each DMA descriptor is >=4KB of
    # contiguous DRAM per partition (256B descriptors are below the SDMA
    # line-rate minimum and take an HBM read-modify-write penalty).
    xh_d = nc.dram_tensor(
        "xh", [B_PER, SLABS, P, KK, SEQ], f16, kind="ExternalInput").ap()
    bh_d = nc.dram_tensor(
        "bh", [B_PER, P, KI, RPAD], f16, kind="ExternalInput").ap()
    ah_d = nc.dram_tensor(
        "ah", [B_PER, RPAD, DOUT], f16, kind="ExternalInput").ap()
    out = nc.dram_tensor("out", [B_PER, SEQ, DOUT], f16, kind="ExternalOutput").ap()

    with tile.TileContext(nc) as tc, ExitStack() as ctx:
        adp = ctx.enter_context(tc.tile_pool(name="adp", bufs=2))
        xhp = ctx.enter_context(tc.tile_pool(name="xhp", bufs=4))
        bxsp = ctx.enter_context(tc.tile_pool(name="bxsp", bufs=2))
        stg = ctx.enter_context(tc.tile_pool(name="stg", bufs=3))
        bxp = ctx.enter_context(tc.tile_pool(name="bxp", bufs=NB, space="PSUM"))
        outp = ctx.enter_context(tc.tile_pool(name="outp", bufs=2, space="PSUM"))

        def load_adapters(s):
            ad = {}
            bht = adp.tile([P, KI, RPAD], f16, name="bh", tag="bh")
            nc.sync.dma_start(bht[:], bh_d[s])
            ad["bh"] = bht
            aht = adp.tile([RPAD, DOUT], f16, name="ah", tag="ah")
            nc.sync.dma_start(aht[:], ah_d[s])
            ad["ah"] = aht
            return ad

        def mm1_slab(s, j, ad, bx_ps):
            """Load x slab j and run its mm1 matmuls."""
            xht = xhp.tile([P, KK, SEQ], f16, name="xht", tag="xht")
            nc.sync.dma_start(xht[:], xh_d[s, j])
            for kk in range(KK):
                k = j * KK + kk
                for nb in range(NB):
                    mv = slice(nb * 512, (nb + 1) * 512)
                    nc.tensor.matmul(bx_ps[nb][:], ad["bh"][:, k, :],
                                     xht[:, kk, mv],
                                     start=(k == 0), stop=(k == KI - 1))

        def bx_drain(bx_ps):
            """Drain mm1 PSUM to fp16 SBUF."""
            bxh = bxsp.tile([RPAD, SEQ], f16, name="bxh", tag="bxh")
            for nb in range(NB):
                sl = slice(nb * 512, (nb + 1) * 512)
                if nb % 2 == 0:
                    nc.vector.tensor_copy(bxh[:, sl], bx_ps[nb][:])
                else:
                    nc.scalar.copy(bxh[:, sl], bx_ps[nb][:])
            return bxh

        def mm2_block(s, nsp, ad, bxh):
            """Two 128-row output blocks: 16 matmuls + drains + one store."""
            st = stg.tile([P, 2, DOUT], f16, name="st", tag="st")
            for c in range(2):
                ns = 2 * nsp + c
                lh = slice(ns * P, (ns + 1) * P)
                for otp in range(OT // 2):  # pairs of 512-col blocks
                    ps = outp.tile([P, 1024], f32, name="ps_o", tag="ps_o")
                    for half in range(2):
                        ot = otp * 2 + half
                        ov = slice(ot * 512, (ot + 1) * 512)
                        pv = slice(half * 512, (half + 1) * 512)
                        nc.tensor.matmul(ps[:, pv], bxh[:, lh], ad["ah"][:, ov],
                                         start=True, stop=True)
                    dv = slice(otp * 1024, (otp + 1) * 1024)
                    if otp % 2 == 0:
                        nc.vector.tensor_copy(st[:, c, dv], ps[:])
                    else:
                        nc.scalar.copy(st[:, c, dv], ps[:])
            nc.gpsimd.dma_start(
                out[s, nsp * 2 * P:(nsp + 1) * 2 * P, :].rearrange(
                    "(c p) o -> p c o", p=P),
                st[:])

        def mm1_sample(s, ad):
            bx_ps = [bxp.tile([P, 512], f32, name="bx_ps", tag="bx_ps")
                     for _ in range(NB)]
            for j in range(SLABS):
                mm1_slab(s, j, ad, bx_ps)
            return bx_drain(bx_ps)

        samples = [s for _ in range(repeat) for s in range(B_PER)]
        # software pipeline: mm1(s0); then interleave mm2(s_i) with mm1(s_{i+1})
        ad_cur = load_adapters(samples[0])
        bxh = mm1_sample(samples[0], ad_cur)
        for idx, s in enumerate(samples):
            nxt = samples[idx + 1] if idx + 1 < len(samples) else None
            if nxt is not None:
                ad_nxt = load_adapters(nxt)
                bx_ps_n = [bxp.tile([P, 512], f32, name="bx_ps", tag="bx_ps")
                           for _ in range(NB)]
                for nsp in range(NSUB // 2):
                    mm2_block(s, nsp, ad_cur, bxh)
                    mm1_slab(nxt, nsp, ad_nxt, bx_ps_n)
                bxh = bx_drain(bx_ps_n)
                ad_cur = ad_nxt
            else:
                for nsp in range(NSUB // 2):
                    mm2_block(s, nsp, ad_cur, bxh)
    nc.compile()
    return nc


def _get_nc(repeat=1):
    key = f"nc{repeat}"
    if key not in _CACHE:
        _CACHE[key] = _build_nc(repeat)
    return _CACHE[key]


def _prep_in_maps(x, adapter_ids, A, B):
    x = np.asarray(x, dtype=np.float32)
    ids = np.asarray(adapter_ids).astype(np.int64)
    A = np.asarray(A, dtype=np.float32)
    B = np.asarray(B, dtype=np.float32)

    As = A * SCALE
    in_maps = []
    for c in range(NCORES):
        sl = slice(c * B_PER, (c + 1) * B_PER)
        cids = ids[sl]
        xT = x[sl].transpose(0, 2, 1).astype(np.float16)          # [2, DIN, SEQ]
        # [2, SLABS, P, KK, SEQ]: DIN row j*KK*P + kk*P + p -> [j, p, kk]
        xT = np.ascontiguousarray(
            xT.reshape(B_PER, SLABS, KK, P, SEQ).transpose(0, 1, 3, 2, 4))
        BT = np.zeros((B_PER, DIN, RPAD), np.float16)
        BT[:, :, :RANK] = B[cids].transpose(0, 2, 1)
        # [2, P, KI, RPAD]: DIN row k*P + p -> [p, k]
        BT = np.ascontiguousarray(
            BT.reshape(B_PER, KI, P, RPAD).transpose(0, 2, 1, 3))
        AT = np.zeros((B_PER, RPAD, DOUT), np.float16)
        AT[:, :RANK, :] = As[cids].transpose(0, 2, 1)
        in_maps.append({"xh": xT, "bh": BT, "ah": AT})
    return in_maps


def kernel(x, adapter_ids, A, B):
    nc = _get_nc()
    in_maps = _prep_in_maps(x, adapter_ids, A, B)
    res = bass_utils.run_bass_kernel_spmd(
        nc, in_maps, core_ids=list(range(NCORES)))
    out = np.empty((BATCH, SEQ, DOUT), dtype=np.float32)
    for c in range(NCORES):
        out[c * B_PER:(c + 1) * B_PER] = res.results[c]["out"].astype(np.float32)
    return out
